# revision 40
# baseline (speedup 1.0000x reference)
"""Trainium2 Bass kernel: per-species expert linear + structure segment-sum.

Math: out[g] = sum_{atoms i in structure g} (x[i] @ W[species_i] + b[species_i])
Since everything is linear, aggregate first, matmul after:
  A[g, s, :] = sum_{i: struct_i=g, species_i=s} x[i]                 (256-dim)
  out[g]     = sum_s A[g, s, :] @ W_s  (+ count[g,s] * b_s, on host)

Stage 1 (on device): segment-sum of x rows by seg' = 256*species +
struct_local, via one-hot matmuls on the TensorEngine accumulating in PSUM.
Atoms are re-sorted per core by (species, struct) on the host, so a
128-atom tile touches only ~6 consecutive seg' values: each tile gets ONE
narrow mask (DVE tensor_scalar, width mostly 32) and one matmul of M rows
at psum base in {0,32,64} of its 128-seg window ((base,maxM) allowed by the
PE: (0,<=128),(32,32),(64,64)). Rare tiles straddle into the next window
(second M=64 matmul; the straddle segs ride in the same mask's trailing
columns, which is why straddlers force base=64/width=128). PSUM windows are
pre-zeroed by a zero-mask matmul when they open, so all stage-1 matmuls
accumulate with start=False. The tile schedule comes from the actual
indices, unioned across all 8 cores (identical SPMD graph).

Stage 2 (on device): because seg' is species-major with species blocks
padded to 256, every 128-seg window is exactly (species s = w//2, structs
(w%2)*128..+128). At each window flush: PE-transpose the accumulator into
feature-major tk buffers, then TWO contiguous M=128 matmuls (one per
128-feature chunk) accumulate W_s into the po psum for that struct half.
This streams the expert weights once per window (16 matmuls total) instead
of 3x with the old struct-major interleaved layout. The bias term
(count[g,s] * b_s) is added on the host. x streams in bf16 (one-hot exact
in bf16; rel err ~3e-3), PSUM accumulates f32.

Sharding: 25000 contiguous atoms per core (structs stay contiguous because
structural_indices are sorted); host overlap-adds the 8 partial per-struct
outputs. x is packed chunk-contiguous (each DMA src one linear DRAM block),
issued alternately from the Sync and Scalar HWDGE queues; const tensors go
via the idle GpSimd SWDGE queue. The first DMA carries segs + a bf16 iota
row + x tile 0 (bitcast-packed into one f32 tensor) so the mask pipeline
and the PE start as early as possible.
"""

import numpy as np

P = 128
N_ATOMS = 200_000
D_IN = 256
D_OUT = 256
N_SPECIES = 4
N_STRUCT = 2_000
N_CORES = 8
SH = N_ATOMS // N_CORES            # atoms per core
TPC = (SH + P - 1) // P            # tiles per core
SH_PAD = TPC * P                   # padded atoms per core
CH = 7                             # max x tiles per DMA chunk
CHUNK_BUFS = 12
M_BUFS = 48
AT_BUFS = 2
TP_BUFS = 2
DF = D_IN                          # features per tile (bias handled on host)
SENTINEL = 3.0e8                   # seg value for padded atoms (matches nothing)
SEGC = TPC + 64 + 128              # segs tensor cols: segs | iota | x tile 0
NW = 8                             # 4 species x 256 struct slots / 128


def _chunk_plan():
    """Chunk start tiles (tile 0 rides in the segs DMA): small head for fast
    pipeline fill, CH-sized body, small tail."""
    starts = [1, 2, 4]
    t = 7
    while t + CH <= TPC - 7:
        starts.append(t)
        t += CH
    while t < TPC:
        starts.append(t)
        t += 2
    sizes = {s: (starts + [TPC])[i + 1] - s for i, s in enumerate(starts)}
    return starts, sizes


def _schedule(seg_local_real):
    """seg_local_real: list of per-core int arrays [SH] of local seg' ids
    (256*species + struct_local, atoms sorted by seg').
    Per-tile schedule, unioned across cores (identical SPMD graph)."""
    assert max(int(s.max()) for s in seg_local_real) < NW * P

    mn = np.full(TPC, 1 << 30, np.int64)
    mx = np.full(TPC, -1, np.int64)
    for s in seg_local_real:
        for t in range(TPC):
            a0, a1 = t * P, min((t + 1) * P, SH)
            if a0 >= SH:
                break
            tl = s[a0:a1]
            mn[t] = min(mn[t], int(tl.min()))
            mx[t] = max(mx[t], int(tl.max()))

    # PE psum-write constraint: base 0/32/64, M <= {128,32,64} resp.
    win = [int(m) // P for m in mn]
    base = []
    mm = []
    strad = []
    for t in range(TPC):
        lo = int(mn[t]) - P * win[t]
        hi = int(mx[t]) - P * win[t]
        strad.append(hi >= P)
        if strad[t]:
            assert lo >= 64, "straddling tile not in upper half-window"
            assert int(mx[t]) - P * (win[t] + 1) < 64, "straddle exceeds M=64"
            base.append(64)
            mm.append(64)
            continue
        b = min(64, lo // 32 * 32)
        if b == 32 and hi >= 64:
            b = 0
        base.append(b)
        mm.append(min(128 if b == 0 else b, (hi - b) // 32 * 32 + 32))
        assert b + mm[t] > hi
    win_first = {}
    win_last = {}
    for t in range(TPC):
        for w in ([win[t], win[t] + 1] if strad[t] else [win[t]]):
            win_first.setdefault(w, t)
            win_last[w] = t
    assert sorted(win_first) == list(range(NW)), "all 8 windows must be live"
    alive = max(
        sum(1 for w in win_first if win_first[w] <= t <= win_last[w])
        for t in range(TPC)
    )
    win_bufs = min(max(2, alive + 1), 4)
    return {
        "win": win,
        "base": base,
        "mm": mm,
        "strad": strad,
        "win_first": win_first,
        "win_last": win_last,
        "win_bufs": win_bufs,
    }


def _build(sched, reps=1):
    import contextlib

    import concourse.bacc as bacc
    import concourse.mybir as mybir
    import concourse.tile as tile

    f32 = mybir.dt.float32
    bf16 = mybir.dt.bfloat16
    starts, sizes = _chunk_plan()
    nchunks = len(starts)

    nc = bacc.Bacc(None, target_bir_lowering=False)
    xp_d = nc.declare_dram_parameter("xp", [nchunks * P, CH * DF], bf16, isOutput=False)
    segs_d = nc.declare_dram_parameter("segs", [P, SEGC], f32, isOutput=False)
    wk_d = nc.declare_dram_parameter("wk", [P, 8 * D_OUT], bf16, isOutput=False)
    id_d = nc.declare_dram_parameter("ident", [P, P], bf16, isOutput=False)
    out_d = nc.declare_dram_parameter("out", [2 * P, D_OUT], f32, isOutput=True)

    with tile.TileContext(nc) as tc:
        with (
            tc.tile_pool(name="const", bufs=1) as constp,
            tc.tile_pool(name="chunk", bufs=CHUNK_BUFS) as chunkp,
            tc.tile_pool(name="onehot", bufs=M_BUFS) as mp,
            tc.tile_pool(name="atmp", bufs=AT_BUFS) as atp,
            tc.tile_pool(name="tks", bufs=1) as tkp,
            tc.tile_pool(name="win", bufs=sched["win_bufs"], space="PSUM") as winp,
            tc.tile_pool(name="tp", bufs=TP_BUFS, space="PSUM") as tpp,
            tc.tile_pool(name="po", bufs=2, space="PSUM") as pop,
        ):
            # one first DMA: segs | bf16 iota (as f32 pairs) | bf16 x tile 0
            segio_sb = constp.tile([P, SEGC], f32)
            nc.sync.dma_start(segio_sb[:], segs_d[:])
            segs_sb = segio_sb
            iota_sb = segio_sb[:, TPC : TPC + 64].bitcast(bf16)
            xt0 = segio_sb[:, TPC + 64 : TPC + 192].bitcast(bf16)
            ident_sb = constp.tile([P, P], bf16)
            wk_sb = constp.tile([P, 8 * D_OUT], bf16)
            zmask_sb = constp.tile([P, P], bf16)
            scratch_sb = constp.tile([1, 1], f32)

            tk0 = tkp.tile([P, NW * P], bf16, tag="tk0")
            tk1 = tkp.tile([P, NW * P], bf16, tag="tk1")

            loop_cm = (
                tc.For_i(
                    0,
                    reps,
                    1,
                    hint_engines=(
                        mybir.EngineType.PE,
                        mybir.EngineType.DVE,
                        mybir.EngineType.Activation,
                        mybir.EngineType.SP,
                    ),
                )
                if reps > 1
                else contextlib.nullcontext()
            )
            first_body = [True]
            with loop_cm:
                _emit_body(
                    nc, tc, mybir, f32, bf16, sched, starts, sizes,
                    chunkp, mp, atp, winp, tpp, pop,
                    segs_sb, iota_sb, xt0, ident_sb, wk_sb, zmask_sb,
                    scratch_sb, tk0, tk1, xp_d, out_d, id_d, wk_d, first_body,
                )

    nc.compile()
    return nc


def _emit_body(
    nc, tc, mybir, f32, bf16, sched, starts, sizes,
    chunkp, mp, atp, winp, tpp, pop,
    segs_sb, iota_sb, xt0, ident_sb, wk_sb, zmask_sb,
    scratch_sb, tk0, tk1, xp_d, out_d, id_d, wk_d, first_body,
):
    win = sched["win"]
    base = sched["base"]
    mm = sched["mm"]
    strad = sched["strad"]
    win_first = sched["win_first"]
    win_last = sched["win_last"]

    po_tiles = {}

    def emit_stage2_window(w):
        # window w = (species w//2, structs (w%2)*128..+128): two contiguous
        # M=128 matmuls accumulate W_s into the po psum for struct half w%2
        s, G = w // 2, w % 2
        if G not in po_tiles:
            po_tiles[G] = pop.tile([P, D_OUT], f32, tag="po", name=f"po{G}")
        po = po_tiles[G]
        for kc, tkbuf in ((0, tk0), (1, tk1)):
            nc.tensor.matmul(
                po[:],
                lhsT=tkbuf[:, w * P : (w + 1) * P],
                rhs=wk_sb[:, (s * 2 + kc) * D_OUT : (s * 2 + kc + 1) * D_OUT],
                start=(w < 2 and kc == 0),
                stop=(w >= 6 and kc == 1),
            )
        if w >= 6:
            ob = atp.tile([P, D_OUT], f32, tag="ob", name=f"ob{G}")
            nc.scalar.copy(ob[:], po[:])
            nc.sync.dma_start(out_d[G * P : (G + 1) * P, :], ob[:])
            del po_tiles[G]

    psw = {}
    chunk = None
    coff = 0
    ci = -1
    for t in range(TPC):
        if t == 0 and first_body[0]:
            # no deps -> runs during the NEFF preamble, ahead of the first
            # (segs-gated) mask op on the DVE queue
            nc.vector.memset(zmask_sb[:], 0.0)
        if t in sizes:
            ci += 1
            csz = sizes[t]
            chunk = chunkp.tile([P, CH * DF], bf16, tag="chunk", name=f"ch{t}")
            eng = nc.sync if ci % 2 == 0 else nc.scalar
            eng.dma_start(
                chunk[:, : csz * DF], xp_d[ci * P : (ci + 1) * P, : csz * DF]
            )
            coff = t
            if first_body[0]:
                if ci == 1:
                    # consts go via the idle GpSimd SWDGE queue so the two
                    # HWDGE queues carry nothing but the x stream
                    nc.gpsimd.dma_start(ident_sb[:], id_d[:])
                    nc.gpsimd.dma_start(wk_sb[:], wk_d[:])
                elif ci == 2:
                    # trigger the scalar-engine act table load early (1.3us)
                    # so it doesn't stall the first window flush
                    nc.scalar.copy(scratch_sb[:], segs_sb[:1, :1])
                    first_body[0] = False
        xt = xt0 if t == 0 else chunk[:, (t - coff) * DF : (t - coff + 1) * DF]
        m = mp.tile([P, P], bf16, tag="m")
        # m[a, j] = (iota[j] == seg[a] - (128w+base)); straddling tiles use
        # the full 128 cols (cols >= 64 encode the next window's segs)
        mw = P if strad[t] else mm[t]
        nc.vector.tensor_scalar(
            out=m[:, :mw],
            in0=iota_sb[:, :mw],
            scalar1=segs_sb[:, t : t + 1],
            scalar2=None,
            op0=mybir.AluOpType.is_equal,
        )
        targets = [(win[t], base[t], 0, mm[t])]
        if strad[t]:
            targets.append((win[t] + 1, 0, 64, 64))
        for w, bs, mo, mwid in targets:
            if w not in psw:
                psw[w] = winp.tile([P, DF], f32, tag="win", name=f"win{w}")
                # pre-zero the whole window so stage-1 accumulates freely
                nc.tensor.matmul(
                    psw[w][:], lhsT=zmask_sb[:], rhs=xt, start=True, stop=False,
                    skip_group_check=True,
                )
            nc.tensor.matmul(
                psw[w][bs : bs + mwid, :],
                lhsT=m[:, mo : mo + mwid],
                rhs=xt,
                start=False,
                stop=(t == win_last[w]),
                skip_group_check=True,
            )
        # flush finished windows: transpose into feature-major buffers
        for w in sorted(psw):
            if t != win_last[w]:
                continue
            at = atp.tile([P, DF], bf16, tag="at")
            nc.scalar.copy(at[:], psw[w][:])
            for kc, tkbuf in ((0, tk0), (1, tk1)):
                tp = tpp.tile([P, P], bf16, tag="tp")
                nc.tensor.transpose(
                    out=tp[:],
                    in_=at[:, kc * P : (kc + 1) * P],
                    identity=ident_sb[:],
                )
                nc.scalar.copy(tkbuf[:, w * P : (w + 1) * P], tp[:])
            del psw[w]
            emit_stage2_window(w)


def _prep(x, W, b, central_species, structural_indices):
    """Host-side prep: schedule from indices + packed per-core in_maps."""
    import ml_dtypes

    bf16 = ml_dtypes.bfloat16
    x = np.asarray(x, dtype=np.float32)
    Wf = np.asarray(W, dtype=np.float32)
    bf = np.asarray(b, dtype=np.float32)
    cs = np.asarray(central_species).astype(np.int64)
    si = np.asarray(structural_indices).astype(np.int64)

    if not np.all(np.diff(si) >= 0):
        order = np.argsort(si, kind="stable")
        si = si[order]
        cs = cs[order]
        x = x[order]

    # host-side bias term: sum over atoms of b[species], per structure
    counts = np.bincount(4 * si + cs, minlength=4 * N_STRUCT).reshape(N_STRUCT, 4)
    bias_full = counts.astype(np.float32) @ bf

    # per-core species-major resort: seg' = 256*species + struct_local
    g0 = [int(si[c * SH]) for c in range(N_CORES)]
    seg_local_real = []
    xs = []
    for c in range(N_CORES):
        sl = si[c * SH : (c + 1) * SH] - g0[c]
        assert int(sl.max()) < 256, "core spans >256 structures"
        sp = 256 * cs[c * SH : (c + 1) * SH] + sl
        order = np.argsort(sp, kind="stable")
        seg_local_real.append(sp[order])
        xs.append(x[c * SH : (c + 1) * SH][order])
    sched = _schedule(seg_local_real)
    starts, sizes = _chunk_plan()

    # bf16 iota row 0..127, shipped packed inside the f32 segs tensor
    iota_f32 = np.ascontiguousarray(
        np.tile(np.arange(P, dtype=bf16), (P, 1))
    ).view(np.float32)
    badj = (
        P * np.asarray(sched["win"], np.float32)
        + np.asarray(sched["base"], np.float32)
    )[None, :]  # [1, TPC]
    ident = np.eye(P, dtype=bf16)
    wk = np.zeros((P, 8, D_OUT), bf16)
    for s in range(N_SPECIES):
        for kc in range(2):
            wk[:, s * 2 + kc, :] = Wf[s, kc * P : (kc + 1) * P, :].astype(bf16)
    wk = np.ascontiguousarray(wk.reshape(P, 8 * D_OUT))

    in_maps = []
    for c in range(N_CORES):
        xp = np.zeros((SH_PAD, DF), bf16)
        xp[:SH] = xs[c].astype(bf16)
        # partition-major within each chunk; chunks are contiguous DRAM
        # blocks so every DMA src is one linear region; tile 0 rides in the
        # segs tensor
        xp = xp.reshape(TPC, P, DF)
        xpk = np.zeros((len(starts) * P, CH * DF), bf16)
        for ci, t0 in enumerate(starts):
            csz = sizes[t0]
            blk = xp[t0 : t0 + csz].transpose(1, 0, 2).reshape(P, csz * DF)
            xpk[ci * P : (ci + 1) * P, : csz * DF] = blk
        segsT = np.full((TPC, P), SENTINEL, np.float32)
        segsT.reshape(-1)[:SH] = seg_local_real[c].astype(np.float32)
        # pre-subtract each tile's window/base offset so the device mask op
        # is a single is_equal against the iota row
        segsT = segsT.T - badj
        xt0_f32 = np.ascontiguousarray(xp[0]).view(np.float32)
        segsT = np.ascontiguousarray(
            np.concatenate([segsT, iota_f32, xt0_f32], axis=1)
        )
        in_maps.append({"xp": xpk, "segs": segsT, "wk": wk, "ident": ident})
    return {
        "build_args": (sched,),
        "in_maps": in_maps,
        "g0": g0,
        "bias_full": bias_full,
    }


def kernel(x, W, b, central_species, structural_indices):
    from concourse.bass_utils import run_bass_kernel_spmd

    prep = _prep(x, W, b, central_species, structural_indices)
    nc = _build(*prep["build_args"])
    res = run_bass_kernel_spmd(
        nc, prep["in_maps"], core_ids=list(range(N_CORES))
    )

    g0 = prep["g0"]
    full = np.zeros((N_STRUCT + 2 * P, D_OUT), np.float32)
    for c in range(N_CORES):
        full[g0[c] : g0[c] + 2 * P] += res.results[c]["out"]
    out = full[:N_STRUCT] + prep["bias_full"]
    return np.ascontiguousarray(out)


# revision 44
# speedup vs baseline: 1.0776x; 1.0776x over previous
"""Trainium2 Bass kernel: per-species expert linear + structure segment-sum.

Math: out[g] = sum_{atoms i in structure g} (x[i] @ W[species_i] + b[species_i])
Since everything is linear, aggregate first, matmul after:
  A[g, s, :] = sum_{i: struct_i=g, species_i=s} [x[i] | 1]        (257-dim)
  out[g]     = sum_s A[g, s, :] @ [[W_s], [b_s]]                  (257 x 256)

Stage 1 (on device): segment-sum of [x | 1] rows by combined seg = 4*struct +
species, via one-hot matmuls on the TensorEngine accumulating in PSUM.
Atoms are pre-sorted by struct, so a 128-atom tile touches only ~8-20 segs;
masks are built at 64-seg-block granularity (DVE tensor_scalar of [128, W]
with W = 64 or 128) and each touched 64-block gets its own M=64 matmul into
the 128-seg PSUM window (out partition base 0/64). This cuts the DVE mask
cost vs full 128-wide masks -- DVE is the bottleneck engine. The
tile->block schedule comes from the actual indices, unioned across all 8
cores so the SPMD graph is identical on every core.

Stage 2 (on device): transpose window accumulators (PE transpose) and
contract the 257-dim feature axis against the packed expert weights,
emitted per pair of windows as soon as they are flushed. x streams in bf16
(one-hot is exact in bf16; rel err ~3e-3 total), PSUM accumulates f32.

Sharding: 25000 contiguous atoms per core (structs stay contiguous per core
because structural_indices are sorted); host overlap-adds the 8 partial
per-struct outputs. x is packed chunk-contiguous on host (each DMA src is
one linear DRAM block) and the x stream is issued alternately from the Sync
and Scalar HWDGE queues so descriptor generation never serializes.
"""

import numpy as np

P = 128
N_ATOMS = 200_000
D_IN = 256
D_OUT = 256
N_SPECIES = 4
N_STRUCT = 2_000
N_CORES = 8
SH = N_ATOMS // N_CORES            # atoms per core
TPC = (SH + P - 1) // P            # tiles per core
SH_PAD = TPC * P                   # padded atoms per core
CH = 7                             # max x tiles per DMA chunk
CHUNK_BUFS = 8
M_BUFS = 12
AT_BUFS = 2
TP_BUFS = 2
PO_BUFS = 2
DF = D_IN + 1                      # features + ones column
SENTINEL = 3.0e8                   # seg value for padded atoms (matches nothing)
B = 64                             # seg block granularity (psum bases 0/64 only)


def _chunk_plan():
    """Chunk start tiles: small head (fast pipeline fill), CH-sized body,
    small tail (short serial tail)."""
    starts = [0, 2, 4]
    t = 7
    while t + CH <= TPC - 7:
        starts.append(t)
        t += CH
    while t < TPC:
        starts.append(t)
        t += 2
    sizes = {s: (starts + [TPC])[i + 1] - s for i, s in enumerate(starts)}
    return starts, sizes


def _schedule(seg_local_real):
    """seg_local_real: list of per-core int arrays [SH] of local seg ids.
    Block-granular schedule, unioned across cores (identical SPMD graph).
    Returns dict with per-tile block ranges, per-block first/last tiles,
    window flush schedule, and PSUM pool sizing."""
    max_seg = max(int(s.max()) for s in seg_local_real)
    n_blocks = max_seg // B + 1
    NW = ((n_blocks + 1) // 2 + 3) // 4 * 4  # windows of 2 blocks, mult of 4

    b0 = np.full(TPC, 1 << 30, np.int64)
    b1 = np.full(TPC, -1, np.int64)
    for s in seg_local_real:
        for t in range(TPC):
            a0, a1 = t * P, min((t + 1) * P, SH)
            if a0 >= SH:
                break
            tl = s[a0:a1]
            b0[t] = min(b0[t], int(tl.min()) // B)
            b1[t] = max(b1[t], int(tl.max()) // B)
    assert int((b1 - b0).max()) < 2, "tile spans >2 seg blocks"

    first_b = {}
    last_b = {}
    for t in range(TPC):
        for b in range(int(b0[t]), int(b1[t]) + 1):
            if b not in first_b:
                first_b[b] = t
            last_b[b] = t

    win_first = {}
    win_last = {}
    for b in first_b:
        w = b // 2
        win_first[w] = min(win_first.get(w, 1 << 30), first_b[b])
        win_last[w] = max(win_last.get(w, -1), last_b[b])
    # untouched blocks inside touched windows -> zeroed at flush
    zero_blocks = {
        w: [b for b in range(2 * w, 2 * w + 2) if b not in first_b]
        for w in win_first
    }
    alive = max(
        sum(1 for w in win_first if win_first[w] <= t <= win_last[w])
        for t in range(TPC)
    )
    win_bufs = min(max(2, alive + 1), 4)
    return {
        "NW": NW,
        "b0": [int(v) for v in b0],
        "b1": [int(v) for v in b1],
        "first_b": first_b,
        "last_b": last_b,
        "win_first": win_first,
        "win_last": win_last,
        "zero_blocks": zero_blocks,
        "win_bufs": win_bufs,
    }


def _build(sched, reps=1):
    import contextlib

    import concourse.bacc as bacc
    import concourse.mybir as mybir
    import concourse.tile as tile

    f32 = mybir.dt.float32
    bf16 = mybir.dt.bfloat16
    NW = sched["NW"]
    starts, sizes = _chunk_plan()
    nchunks = len(starts)

    nc = bacc.Bacc(None, target_bir_lowering=False)
    xp_d = nc.declare_dram_parameter("xp", [nchunks * P, CH * DF], bf16, isOutput=False)
    segs_d = nc.declare_dram_parameter("segs", [P, TPC + P], f32, isOutput=False)
    wk_d = nc.declare_dram_parameter("wk", [P, 8 * D_OUT], bf16, isOutput=False)
    wb_d = nc.declare_dram_parameter("wb", [1, N_SPECIES * D_OUT], bf16, isOutput=False)
    id_d = nc.declare_dram_parameter("ident", [P, P], bf16, isOutput=False)
    out_d = nc.declare_dram_parameter("out", [NW * 32, D_OUT], f32, isOutput=True)

    with tile.TileContext(nc) as tc:
        with (
            tc.tile_pool(name="const", bufs=1) as constp,
            tc.tile_pool(name="chunk", bufs=CHUNK_BUFS) as chunkp,
            tc.tile_pool(name="onehot", bufs=M_BUFS) as mp,
            tc.tile_pool(name="atmp", bufs=AT_BUFS) as atp,
            tc.tile_pool(name="tks", bufs=1) as tkp,
            tc.tile_pool(name="win", bufs=sched["win_bufs"], space="PSUM") as winp,
            tc.tile_pool(name="tp", bufs=TP_BUFS, space="PSUM") as tpp,
            tc.tile_pool(name="po", bufs=PO_BUFS, space="PSUM") as pop,
        ):
            # segs columns 0:TPC, f32 iota columns TPC:TPC+P, one DMA (first)
            segio_sb = constp.tile([P, TPC + P], f32)
            nc.sync.dma_start(segio_sb[:], segs_d[:])
            segs_sb = segio_sb
            iota_bf = constp.tile([P, P], bf16)
            nc.vector.tensor_copy(iota_bf[:], segio_sb[:, TPC : TPC + P])
            iota_sb = iota_bf[:]
            ident_sb = constp.tile([P, P], bf16)
            wk_sb = constp.tile([P, 8 * D_OUT], bf16)
            wb_sb = constp.tile([1, N_SPECIES * D_OUT], bf16)
            zmask_sb = constp.tile([P, B], bf16)
            scratch_sb = constp.tile([1, 1], f32)

            tk0 = tkp.tile([P, NW * P], bf16, tag="tk0")
            tk1 = tkp.tile([P, NW * P], bf16, tag="tk1")
            tb = tkp.tile([1, NW * P], bf16, tag="tb")

            loop_cm = (
                tc.For_i(
                    0,
                    reps,
                    1,
                    hint_engines=(
                        mybir.EngineType.PE,
                        mybir.EngineType.DVE,
                        mybir.EngineType.Activation,
                        mybir.EngineType.SP,
                    ),
                )
                if reps > 1
                else contextlib.nullcontext()
            )
            first_body = [True]
            with loop_cm:
                _emit_body(
                    nc, tc, mybir, f32, bf16, sched, starts, sizes,
                    chunkp, mp, atp, winp, tpp, pop,
                    segs_sb, iota_sb, ident_sb, wk_sb, wb_sb, zmask_sb,
                    scratch_sb, tk0, tk1, tb, xp_d, out_d, id_d, wk_d, wb_d,
                    first_body,
                )

    nc.compile()
    return nc


def _emit_body(
    nc, tc, mybir, f32, bf16, sched, starts, sizes,
    chunkp, mp, atp, winp, tpp, pop,
    segs_sb, iota_sb, ident_sb, wk_sb, wb_sb, zmask_sb,
    scratch_sb, tk0, tk1, tb, xp_d, out_d, id_d, wk_d, wb_d, first_body,
):
    NW = sched["NW"]
    NWG = NW // 4
    b0 = sched["b0"]
    b1 = sched["b1"]
    first_b = sched["first_b"]
    last_b = sched["last_b"]
    win_last = sched["win_last"]
    zero_blocks = sched["zero_blocks"]

    po_tiles = {}
    po_done = {g: 0 for g in range(NWG)}
    pairs_done = set()

    def emit_stage2_pair(w_lo):
        # windows (w_lo, w_lo+1) fill output partitions [64r, 64r+64) of
        # group g's psum, r = (w_lo//2) % 2 (PE out base must be 0/32/64)
        g, r = w_lo // 4, (w_lo // 2) % 2
        pairs_done.add(w_lo)
        if g not in po_tiles:
            po_tiles[g] = pop.tile([P, D_OUT], f32, tag="po", name=f"po{g}")
        po = po_tiles[g]
        blk = po[64 * r : 64 * r + 64, :]
        for kc, tkbuf in ((0, tk0), (1, tk1)):
            for s in range(N_SPECIES):
                nc.tensor.matmul(
                    blk,
                    lhsT=tkbuf[:, w_lo * P + s : (w_lo + 2) * P : 4],
                    rhs=wk_sb[:, (s * 2 + kc) * D_OUT : (s * 2 + kc + 1) * D_OUT],
                    start=(kc == 0 and s == 0),
                    stop=False,
                )
        for s in range(N_SPECIES):
            nc.tensor.matmul(
                blk,
                lhsT=tb[:1, w_lo * P + s : (w_lo + 2) * P : 4],
                rhs=wb_sb[:1, s * D_OUT : (s + 1) * D_OUT],
                start=False,
                stop=(s == N_SPECIES - 1),
            )
        po_done[g] += 1
        if po_done[g] == 2:
            ob = atp.tile([P, D_OUT], f32, tag="ob", name=f"ob{g}")
            nc.scalar.copy(ob[:], po[:])
            nc.sync.dma_start(out_d[g * P : (g + 1) * P, :], ob[:])
            del po_tiles[g]

    psw = {}
    chunk = None
    coff = 0
    ci = -1
    for t in range(TPC):
        if t in sizes:
            ci += 1
            csz = sizes[t]
            chunk = chunkp.tile([P, CH * DF], bf16, tag="chunk", name=f"ch{t}")
            eng = nc.sync if ci % 2 == 0 else nc.scalar
            eng.dma_start(
                chunk[:, : csz * DF], xp_d[ci * P : (ci + 1) * P, : csz * DF]
            )
            coff = t
            if first_body[0]:
                if ci == 1:
                    # gpsimd is otherwise idle: zero-fill tk + zmask there
                    nc.gpsimd.memset(tk0[:], 0.0)
                    nc.gpsimd.memset(tk1[:], 0.0)
                    nc.gpsimd.memset(tb[:], 0.0)
                    nc.gpsimd.memset(zmask_sb[:], 0.0)
                elif ci == 2:
                    nc.sync.dma_start(ident_sb[:], id_d[:])
                elif ci == 3:
                    # trigger the scalar-engine act table load early (1.3us)
                    # so it doesn't stall the first window flush
                    nc.scalar.copy(scratch_sb[:], segs_sb[:1, :1])
                elif ci == 4:
                    nc.sync.dma_start(wk_sb[:], wk_d[:])
                    nc.sync.dma_start(wb_sb[:], wb_d[:])
                    first_body[0] = False
        xt = chunk[:, (t - coff) * DF : (t - coff + 1) * DF]
        nblk = b1[t] - b0[t] + 1
        m = mp.tile([P, P], bf16, tag="m")
        # m[a, j] = (iota[j] - seg[a] == -64*b0)  <=>  seg[a] == 64*b0 + j
        nc.vector.tensor_scalar(
            out=m[:, : nblk * B],
            in0=iota_sb[:, : nblk * B],
            scalar1=segs_sb[:, t : t + 1],
            scalar2=float(-(B * b0[t])),
            op0=mybir.AluOpType.subtract,
            op1=mybir.AluOpType.is_equal,
        )
        for b in range(b0[t], b1[t] + 1):
            w = b // 2
            if w not in psw:
                psw[w] = winp.tile([P, DF], f32, tag="win", name=f"win{w}")
            base = B * (b % 2)
            nc.tensor.matmul(
                psw[w][base : base + B, :],
                lhsT=m[:, (b - b0[t]) * B : (b - b0[t] + 1) * B],
                rhs=xt,
                start=(t == first_b[b]),
                stop=(t == last_b[b]),
            )
        # flush finished windows: transpose into feature-major buffers
        for w in sorted(psw):
            if t != win_last[w]:
                continue
            for b in zero_blocks[w]:
                base = B * (b % 2)
                nc.tensor.matmul(
                    psw[w][base : base + B, :],
                    lhsT=zmask_sb[:],
                    rhs=xt,
                    start=True,
                    stop=True,
                )
            at = atp.tile([P, DF], bf16, tag="at")
            nc.scalar.copy(at[:], psw[w][:])
            for kc, tkbuf in ((0, tk0), (1, tk1)):
                tp = tpp.tile([P, P], bf16, tag="tp")
                nc.tensor.transpose(
                    out=tp[:],
                    in_=at[:, kc * P : (kc + 1) * P],
                    identity=ident_sb[:],
                )
                nc.scalar.copy(tkbuf[:, w * P : (w + 1) * P], tp[:])
            tpb = tpp.tile([P, P], bf16, tag="tp")
            nc.tensor.transpose(
                out=tpb[:1, :], in_=at[:, D_IN : D_IN + 1], identity=ident_sb[:]
            )
            nc.scalar.copy(tb[:, w * P : (w + 1) * P], tpb[:1, :])
            del psw[w]
            # stage 2 for a window pair once its later window is flushed
            if w % 2 == 1:
                emit_stage2_pair(w - 1)

    # remaining pairs (NW padding / odd tail): zeros via memset tk columns
    for w_lo in range(0, NW, 2):
        if w_lo not in pairs_done:
            emit_stage2_pair(w_lo)


def _prep(x, W, b, central_species, structural_indices):
    """Host-side prep: schedule from indices + packed per-core in_maps."""
    import ml_dtypes

    bf16 = ml_dtypes.bfloat16
    x = np.asarray(x, dtype=np.float32)
    Wf = np.asarray(W, dtype=np.float32)
    bf = np.asarray(b, dtype=np.float32)
    cs = np.asarray(central_species).astype(np.int64)
    si = np.asarray(structural_indices).astype(np.int64)

    if not np.all(np.diff(si) >= 0):
        order = np.argsort(si, kind="stable")
        si = si[order]
        cs = cs[order]
        x = x[order]

    seg = 4 * si + cs
    g0 = [int(si[c * SH]) for c in range(N_CORES)]
    seg_local_real = [
        (seg[c * SH : (c + 1) * SH] - 4 * g0[c]).astype(np.int64)
        for c in range(N_CORES)
    ]
    sched = _schedule(seg_local_real)
    starts, sizes = _chunk_plan()

    iota = np.tile(np.arange(P, dtype=np.float32), (P, 1))
    ident = np.eye(P, dtype=bf16)
    wk = np.zeros((P, 8, D_OUT), bf16)
    for s in range(N_SPECIES):
        for kc in range(2):
            wk[:, s * 2 + kc, :] = Wf[s, kc * P : (kc + 1) * P, :].astype(bf16)
    wk = np.ascontiguousarray(wk.reshape(P, 8 * D_OUT))
    wb = bf.reshape(1, -1).astype(bf16)

    in_maps = []
    for c in range(N_CORES):
        xp = np.zeros((SH_PAD, DF), bf16)
        xp[:SH, :D_IN] = x[c * SH : (c + 1) * SH].astype(bf16)
        xp[:SH, D_IN] = 1.0
        # partition-major within each chunk; chunks are contiguous DRAM
        # blocks so every DMA src is one linear region
        xp = xp.reshape(TPC, P, DF)
        xpk = np.zeros((len(starts) * P, CH * DF), bf16)
        for ci, t0 in enumerate(starts):
            csz = sizes[t0]
            blk = xp[t0 : t0 + csz].transpose(1, 0, 2).reshape(P, csz * DF)
            xpk[ci * P : (ci + 1) * P, : csz * DF] = blk
        segsT = np.full((TPC, P), SENTINEL, np.float32)
        segsT.reshape(-1)[:SH] = seg_local_real[c].astype(np.float32)
        segsT = np.ascontiguousarray(np.concatenate([segsT.T, iota], axis=1))
        in_maps.append(
            {"xp": xpk, "segs": segsT, "wk": wk, "wb": wb, "ident": ident}
        )
    return {
        "build_args": (sched,),
        "in_maps": in_maps,
        "g0": g0,
        "NW": sched["NW"],
    }


def kernel(x, W, b, central_species, structural_indices):
    from concourse.bass_utils import run_bass_kernel_spmd

    prep = _prep(x, W, b, central_species, structural_indices)
    nc = _build(*prep["build_args"])
    res = run_bass_kernel_spmd(
        nc, prep["in_maps"], core_ids=list(range(N_CORES))
    )

    g0, NW = prep["g0"], prep["NW"]
    full = np.zeros((N_STRUCT + NW * 32, D_OUT), np.float32)
    for c in range(N_CORES):
        full[g0[c] : g0[c] + NW * 32] += res.results[c]["out"]
    return np.ascontiguousarray(full[:N_STRUCT])


# revision 67
# speedup vs baseline: 1.0903x; 1.0118x over previous
"""Trainium2 Bass kernel: per-species expert linear + structure segment-sum.

Math: out[g] = sum_{atoms i in structure g} (x[i] @ W[species_i] + b[species_i])
Since everything is linear, aggregate first, matmul after:
  A[g, s, :] = sum_{i: struct_i=g, species_i=s} [x[i] | 1]        (257-dim)
  out[g]     = sum_s A[g, s, :] @ [[W_s], [b_s]]                  (257 x 256)

Stage 1 (on device): segment-sum of [x | 1] rows by combined seg = 4*struct +
species, via one-hot matmuls on the TensorEngine accumulating in PSUM.
Atoms are pre-sorted by struct, so a 128-atom tile touches only ~8-20 segs;
masks are built at 64-seg-block granularity (DVE tensor_scalar of [128, W]
with W = 64 or 128) and each touched 64-block gets its own M=64 matmul into
the 128-seg PSUM window (out partition base 0/64). This cuts the DVE mask
cost vs full 128-wide masks -- DVE is the bottleneck engine. The
tile->block schedule comes from the actual indices, unioned across all 8
cores so the SPMD graph is identical on every core.

Stage 2 (on device): transpose window accumulators (PE transpose) and
contract the 257-dim feature axis against the packed expert weights,
emitted per pair of windows as soon as they are flushed. x streams in bf16
(one-hot is exact in bf16; rel err ~3e-3 total), PSUM accumulates f32.

Sharding: 25000 contiguous atoms per core (structs stay contiguous per core
because structural_indices are sorted); host overlap-adds the 8 partial
per-struct outputs. x is packed chunk-contiguous on host (each DMA src is
one linear DRAM block) and the x stream is issued alternately from the Sync
and Scalar HWDGE queues so descriptor generation never serializes.
"""

import numpy as np

P = 128
N_ATOMS = 200_000
D_IN = 256
D_OUT = 256
N_SPECIES = 4
N_STRUCT = 2_000
N_CORES = 8
SH = N_ATOMS // N_CORES            # atoms per core
TPC = (SH + P - 1) // P            # tiles per core
SH_PAD = TPC * P                   # padded atoms per core
CH = 7                             # max x tiles per DMA chunk
CHUNK_BUFS = 8
M_BUFS = 12
AT_BUFS = 2
TP_BUFS = 2
PO_BUFS = 2
DF = D_IN                          # features per tile (bias handled on host)
SENTINEL = 3.0e8                   # seg value for padded atoms (matches nothing)
B = 64                             # seg block granularity (psum bases 0/64 only)


def _chunk_plan():
    """Chunk start tiles: small head (fast pipeline fill), CH-sized body,
    small tail (short serial tail)."""
    starts = [0, 2, 4]
    t = 7
    while t + CH <= TPC - 7:
        starts.append(t)
        t += CH
    while t < TPC:
        starts.append(t)
        t += 2
    sizes = {s: (starts + [TPC])[i + 1] - s for i, s in enumerate(starts)}
    return starts, sizes


def _schedule(seg_local_real):
    """seg_local_real: list of per-core int arrays [SH] of local seg ids.
    Block-granular schedule, unioned across cores (identical SPMD graph).
    Returns dict with per-tile block ranges, per-block first/last tiles,
    window flush schedule, and PSUM pool sizing."""
    max_seg = max(int(s.max()) for s in seg_local_real)
    n_blocks = max_seg // B + 1
    NW = ((n_blocks + 1) // 2 + 3) // 4 * 4  # windows of 2 blocks, mult of 4

    b0 = np.full(TPC, 1 << 30, np.int64)
    b1 = np.full(TPC, -1, np.int64)
    for s in seg_local_real:
        for t in range(TPC):
            a0, a1 = t * P, min((t + 1) * P, SH)
            if a0 >= SH:
                break
            tl = s[a0:a1]
            b0[t] = min(b0[t], int(tl.min()) // B)
            b1[t] = max(b1[t], int(tl.max()) // B)
    assert int((b1 - b0).max()) < 2, "tile spans >2 seg blocks"

    first_b = {}
    last_b = {}
    for t in range(TPC):
        for b in range(int(b0[t]), int(b1[t]) + 1):
            if b not in first_b:
                first_b[b] = t
            last_b[b] = t

    win_first = {}
    win_last = {}
    for b in first_b:
        w = b // 2
        win_first[w] = min(win_first.get(w, 1 << 30), first_b[b])
        win_last[w] = max(win_last.get(w, -1), last_b[b])
    # untouched blocks inside touched windows -> zeroed at flush
    zero_blocks = {
        w: [b for b in range(2 * w, 2 * w + 2) if b not in first_b]
        for w in win_first
    }
    alive = max(
        sum(1 for w in win_first if win_first[w] <= t <= win_last[w])
        for t in range(TPC)
    )
    win_bufs = min(max(2, alive + 1), 4)
    return {
        "NW": NW,
        "b0": [int(v) for v in b0],
        "b1": [int(v) for v in b1],
        "first_b": first_b,
        "last_b": last_b,
        "win_first": win_first,
        "win_last": win_last,
        "zero_blocks": zero_blocks,
        "win_bufs": win_bufs,
    }


def _build(sched, reps=1):
    import contextlib

    import concourse.bacc as bacc
    import concourse.mybir as mybir
    import concourse.tile as tile

    f32 = mybir.dt.float32
    bf16 = mybir.dt.bfloat16
    NW = sched["NW"]
    starts, sizes = _chunk_plan()
    nchunks = len(starts)

    nc = bacc.Bacc(None, target_bir_lowering=False)
    xp_d = nc.declare_dram_parameter("xp", [nchunks * P, CH * DF], bf16, isOutput=False)
    segs_d = nc.declare_dram_parameter("segs", [P, P + TPC], f32, isOutput=False)
    wk_d = nc.declare_dram_parameter("wk", [P, 8 * D_OUT], bf16, isOutput=False)
    id_d = nc.declare_dram_parameter("ident", [P, P], bf16, isOutput=False)
    out_d = nc.declare_dram_parameter("out", [NW * 32, D_OUT], f32, isOutput=True)

    with tile.TileContext(nc) as tc:
        with (
            tc.tile_pool(name="const", bufs=1) as constp,
            tc.tile_pool(name="chunk", bufs=CHUNK_BUFS) as chunkp,
            tc.tile_pool(name="onehot", bufs=M_BUFS) as mp,
            tc.tile_pool(name="atmp", bufs=AT_BUFS) as atp,
            tc.tile_pool(name="tks", bufs=1) as tkp,
            tc.tile_pool(name="win", bufs=sched["win_bufs"], space="PSUM") as winp,
            tc.tile_pool(name="tp", bufs=TP_BUFS, space="PSUM") as tpp,
            tc.tile_pool(name="po", bufs=PO_BUFS, space="PSUM") as pop,
        ):
            # f32 iota columns 0:P, segs columns P:P+TPC; split into a small
            # fast-completing first DMA (iota + first 32 tile columns, Sync)
            # and the rest on Scalar, so the mask pipeline starts ~1us sooner
            segio_sb = constp.tile([P, P + TPC], f32)
            nc.sync.dma_start(segio_sb[:, : P + 32], segs_d[:, : P + 32])
            nc.scalar.dma_start(segio_sb[:, P + 32 :], segs_d[:, P + 32 :])
            segs_sb = segio_sb
            iota_bf = constp.tile([P, P], bf16)
            nc.vector.tensor_copy(iota_bf[:], segio_sb[:, :P])
            iota_sb = iota_bf[:]
            ident_sb = constp.tile([P, P], bf16)
            wk_sb = constp.tile([P, 8 * D_OUT], bf16)
            zmask_sb = constp.tile([P, B], bf16)
            scratch_sb = constp.tile([1, 1], f32)

            tk0 = tkp.tile([P, NW * P], bf16, tag="tk0")
            tk1 = tkp.tile([P, NW * P], bf16, tag="tk1")

            loop_cm = (
                tc.For_i(
                    0,
                    reps,
                    1,
                    hint_engines=(
                        mybir.EngineType.PE,
                        mybir.EngineType.DVE,
                        mybir.EngineType.Activation,
                        mybir.EngineType.SP,
                    ),
                )
                if reps > 1
                else contextlib.nullcontext()
            )
            first_body = [True]
            with loop_cm:
                _emit_body(
                    nc, tc, mybir, f32, bf16, sched, starts, sizes,
                    chunkp, mp, atp, winp, tpp, pop,
                    segs_sb, iota_sb, ident_sb, wk_sb, zmask_sb,
                    scratch_sb, tk0, tk1, xp_d, out_d, id_d, wk_d,
                    first_body,
                )

    nc.compile()
    return nc


def _emit_body(
    nc, tc, mybir, f32, bf16, sched, starts, sizes,
    chunkp, mp, atp, winp, tpp, pop,
    segs_sb, iota_sb, ident_sb, wk_sb, zmask_sb,
    scratch_sb, tk0, tk1, xp_d, out_d, id_d, wk_d, first_body,
):
    NW = sched["NW"]
    NWG = NW // 4
    b0 = sched["b0"]
    b1 = sched["b1"]
    first_b = sched["first_b"]
    last_b = sched["last_b"]
    win_last = sched["win_last"]
    zero_blocks = sched["zero_blocks"]

    po_tiles = {}
    po_done = {g: 0 for g in range(NWG)}
    pairs_done = set()

    def emit_stage2_pair(w_lo):
        # windows (w_lo, w_lo+1) fill output partitions [64r, 64r+64) of
        # group g's psum, r = (w_lo//2) % 2 (PE out base must be 0/32/64)
        g, r = w_lo // 4, (w_lo // 2) % 2
        pairs_done.add(w_lo)
        if g not in po_tiles:
            po_tiles[g] = pop.tile([P, D_OUT], f32, tag="po", name=f"po{g}")
        po = po_tiles[g]
        blk = po[64 * r : 64 * r + 64, :]
        for kc, tkbuf in ((0, tk0), (1, tk1)):
            for s in range(N_SPECIES):
                nc.tensor.matmul(
                    blk,
                    lhsT=tkbuf[:, w_lo * P + s : (w_lo + 2) * P : 4],
                    rhs=wk_sb[:, (s * 2 + kc) * D_OUT : (s * 2 + kc + 1) * D_OUT],
                    start=(kc == 0 and s == 0),
                    stop=(kc == 1 and s == N_SPECIES - 1),
                )
        po_done[g] += 1
        if po_done[g] == 2:
            ob = atp.tile([P, D_OUT], f32, tag="ob", name=f"ob{g}")
            nc.scalar.copy(ob[:], po[:])
            nc.sync.dma_start(out_d[g * P : (g + 1) * P, :], ob[:])
            del po_tiles[g]

    psw = {}
    chunk = None
    coff = 0
    ci = -1
    for t in range(TPC):
        if t in sizes:
            ci += 1
            csz = sizes[t]
            chunk = chunkp.tile([P, CH * DF], bf16, tag="chunk", name=f"ch{t}")
            eng = nc.sync if ci % 2 == 0 else nc.scalar
            eng.dma_start(
                chunk[:, : csz * DF], xp_d[ci * P : (ci + 1) * P, : csz * DF]
            )
            coff = t
            if first_body[0]:
                if ci == 1:
                    # gpsimd is otherwise idle: zero-fill tk + zmask there
                    nc.gpsimd.memset(tk0[:], 0.0)
                    nc.gpsimd.memset(tk1[:], 0.0)
                    nc.gpsimd.memset(zmask_sb[:], 0.0)
                elif ci == 2:
                    nc.sync.dma_start(ident_sb[:], id_d[:])
                elif ci == 3:
                    # trigger the scalar-engine act table load early (1.3us)
                    # so it doesn't stall the first window flush
                    nc.scalar.copy(scratch_sb[:], segs_sb[:1, :1])
                elif ci == 4:
                    nc.sync.dma_start(wk_sb[:], wk_d[:])
                    first_body[0] = False
        xt = chunk[:, (t - coff) * DF : (t - coff + 1) * DF]
        nblk = b1[t] - b0[t] + 1
        m = mp.tile([P, P], bf16, tag="m")
        # m[a, j] = (iota[j] - seg[a] == -64*b0)  <=>  seg[a] == 64*b0 + j
        nc.vector.tensor_scalar(
            out=m[:, : nblk * B],
            in0=iota_sb[:, : nblk * B],
            scalar1=segs_sb[:, P + t : P + t + 1],
            scalar2=float(-(B * b0[t])),
            op0=mybir.AluOpType.subtract,
            op1=mybir.AluOpType.is_equal,
        )
        for b in range(b0[t], b1[t] + 1):
            w = b // 2
            if w not in psw:
                psw[w] = winp.tile([P, DF], f32, tag="win", name=f"win{w}")
            base = B * (b % 2)
            nc.tensor.matmul(
                psw[w][base : base + B, :],
                lhsT=m[:, (b - b0[t]) * B : (b - b0[t] + 1) * B],
                rhs=xt,
                start=(t == first_b[b]),
                stop=(t == last_b[b]),
            )
        # flush finished windows: transpose into feature-major buffers
        for w in sorted(psw):
            if t != win_last[w]:
                continue
            for b in zero_blocks[w]:
                base = B * (b % 2)
                nc.tensor.matmul(
                    psw[w][base : base + B, :],
                    lhsT=zmask_sb[:],
                    rhs=xt,
                    start=True,
                    stop=True,
                )
            # the last window flushes after DVE's final mask, so its copies
            # run on the then-idle DVE instead of serializing on Scalar
            last_flush = t == TPC - 1
            at = atp.tile([P, DF], bf16, tag="at")
            if last_flush:
                nc.vector.tensor_copy(at[:], psw[w][:])
            else:
                nc.scalar.copy(at[:], psw[w][:])
            for kc, tkbuf in ((0, tk0), (1, tk1)):
                tp = tpp.tile([P, P], bf16, tag="tp")
                nc.tensor.transpose(
                    out=tp[:],
                    in_=at[:, kc * P : (kc + 1) * P],
                    identity=ident_sb[:],
                )
                if last_flush:
                    nc.vector.tensor_copy(tkbuf[:, w * P : (w + 1) * P], tp[:])
                else:
                    nc.scalar.copy(tkbuf[:, w * P : (w + 1) * P], tp[:])
            del psw[w]
            # stage 2 for a window pair once its later window is flushed
            if w % 2 == 1:
                emit_stage2_pair(w - 1)

    # remaining pairs (NW padding / odd tail): zeros via memset tk columns
    for w_lo in range(0, NW, 2):
        if w_lo not in pairs_done:
            emit_stage2_pair(w_lo)


def _prep(x, W, b, central_species, structural_indices):
    """Host-side prep: schedule from indices + packed per-core in_maps."""
    import ml_dtypes

    bf16 = ml_dtypes.bfloat16
    x = np.asarray(x, dtype=np.float32)
    Wf = np.asarray(W, dtype=np.float32)
    bf = np.asarray(b, dtype=np.float32)
    cs = np.asarray(central_species).astype(np.int64)
    si = np.asarray(structural_indices).astype(np.int64)

    if not np.all(np.diff(si) >= 0):
        order = np.argsort(si, kind="stable")
        si = si[order]
        cs = cs[order]
        x = x[order]

    seg = 4 * si + cs
    # host-side bias term: sum over atoms of b[species], per structure
    counts = np.bincount(seg, minlength=4 * N_STRUCT).reshape(N_STRUCT, 4)
    bias_full = counts.astype(np.float32) @ bf
    g0 = [int(si[c * SH]) for c in range(N_CORES)]
    seg_local_real = [
        (seg[c * SH : (c + 1) * SH] - 4 * g0[c]).astype(np.int64)
        for c in range(N_CORES)
    ]
    sched = _schedule(seg_local_real)
    starts, sizes = _chunk_plan()

    iota = np.tile(np.arange(P, dtype=np.float32), (P, 1))
    ident = np.eye(P, dtype=bf16)
    wk = np.zeros((P, 8, D_OUT), bf16)
    for s in range(N_SPECIES):
        for kc in range(2):
            wk[:, s * 2 + kc, :] = Wf[s, kc * P : (kc + 1) * P, :].astype(bf16)
    wk = np.ascontiguousarray(wk.reshape(P, 8 * D_OUT))

    in_maps = []
    for c in range(N_CORES):
        xp = np.zeros((SH_PAD, DF), bf16)
        xp[:SH] = x[c * SH : (c + 1) * SH].astype(bf16)
        # partition-major within each chunk; chunks are contiguous DRAM
        # blocks so every DMA src is one linear region
        xp = xp.reshape(TPC, P, DF)
        xpk = np.zeros((len(starts) * P, CH * DF), bf16)
        for ci, t0 in enumerate(starts):
            csz = sizes[t0]
            blk = xp[t0 : t0 + csz].transpose(1, 0, 2).reshape(P, csz * DF)
            xpk[ci * P : (ci + 1) * P, : csz * DF] = blk
        segsT = np.full((TPC, P), SENTINEL, np.float32)
        segsT.reshape(-1)[:SH] = seg_local_real[c].astype(np.float32)
        segsT = np.ascontiguousarray(np.concatenate([iota, segsT.T], axis=1))
        in_maps.append(
            {"xp": xpk, "segs": segsT, "wk": wk, "ident": ident}
        )
    return {
        "build_args": (sched,),
        "in_maps": in_maps,
        "g0": g0,
        "NW": sched["NW"],
        "bias_full": bias_full,
    }


def kernel(x, W, b, central_species, structural_indices):
    from concourse.bass_utils import run_bass_kernel_spmd

    prep = _prep(x, W, b, central_species, structural_indices)
    nc = _build(*prep["build_args"])
    res = run_bass_kernel_spmd(
        nc, prep["in_maps"], core_ids=list(range(N_CORES))
    )

    g0, NW = prep["g0"], prep["NW"]
    full = np.zeros((N_STRUCT + NW * 32, D_OUT), np.float32)
    for c in range(N_CORES):
        full[g0[c] : g0[c] + NW * 32] += res.results[c]["out"]
    out = full[:N_STRUCT] + prep["bias_full"]
    return np.ascontiguousarray(out)


# revision 81
# speedup vs baseline: 1.1122x; 1.0200x over previous
"""Trainium2 Bass kernel: per-species expert linear + structure segment-sum.

Math: out[g] = sum_{atoms i in structure g} (x[i] @ W[species_i] + b[species_i])
Since everything is linear, aggregate first, matmul after:
  A[g, s, :] = sum_{i: struct_i=g, species_i=s} [x[i] | 1]        (257-dim)
  out[g]     = sum_s A[g, s, :] @ [[W_s], [b_s]]                  (257 x 256)

Stage 1 (on device): segment-sum of [x | 1] rows by combined seg = 4*struct +
species, via one-hot matmuls on the TensorEngine accumulating in PSUM.
Atoms are pre-sorted by struct, so a 128-atom tile touches only ~8-20 segs;
masks are built at 64-seg-block granularity (DVE tensor_scalar of [128, W]
with W = 64 or 128) and each touched 64-block gets its own M=64 matmul into
the 128-seg PSUM window (out partition base 0/64). This cuts the DVE mask
cost vs full 128-wide masks -- DVE is the bottleneck engine. The
tile->block schedule comes from the actual indices, unioned across all 8
cores so the SPMD graph is identical on every core.

Stage 2 (on device): transpose window accumulators (PE transpose) and
contract the 257-dim feature axis against the packed expert weights,
emitted per pair of windows as soon as they are flushed. x streams in bf16
(one-hot is exact in bf16; rel err ~3e-3 total), PSUM accumulates f32.

Sharding: 25000 contiguous atoms per core (structs stay contiguous per core
because structural_indices are sorted); host overlap-adds the 8 partial
per-struct outputs. x is packed chunk-contiguous on host (each DMA src is
one linear DRAM block) and the x stream is issued alternately from the Sync
and Scalar HWDGE queues so descriptor generation never serializes.
"""

import numpy as np

P = 128
N_ATOMS = 200_000
D_IN = 256
D_OUT = 256
N_SPECIES = 4
N_STRUCT = 2_000
N_CORES = 8
SH = N_ATOMS // N_CORES            # atoms per core
TPC = (SH + P - 1) // P            # tiles per core
SH_PAD = TPC * P                   # padded atoms per core
CH = 7                             # max x tiles per DMA chunk
CHUNK_BUFS = 8
M_BUFS = 12
AT_BUFS = 2
TP_BUFS = 2
PO_BUFS = 2
DF = D_IN                          # features per tile (bias handled on host)
SENTINEL = 3.0e8                   # seg value for padded atoms (matches nothing)
B = 64                             # seg block granularity (psum bases 0/64 only)
MASK_FP8 = False                   # fp8e4 masks work (exact) but bench slower


def _chunk_plan():
    """Chunk start tiles: small head (fast pipeline fill), CH-sized body,
    small tail (short serial tail)."""
    starts = [0, 2, 4]
    t = 7
    while t + CH <= TPC - 7:
        starts.append(t)
        t += CH
    while t < TPC:
        starts.append(t)
        t += 2
    sizes = {s: (starts + [TPC])[i + 1] - s for i, s in enumerate(starts)}
    return starts, sizes


def _schedule(seg_local_real):
    """seg_local_real: list of per-core int arrays [SH] of local seg ids.
    Block-granular schedule, unioned across cores (identical SPMD graph).
    Returns dict with per-tile block ranges, per-block first/last tiles,
    window flush schedule, and PSUM pool sizing."""
    max_seg = max(int(s.max()) for s in seg_local_real)
    n_blocks = max_seg // B + 1
    NW = ((n_blocks + 1) // 2 + 3) // 4 * 4  # windows of 2 blocks, mult of 4

    b0 = np.full(TPC, 1 << 30, np.int64)
    b1 = np.full(TPC, -1, np.int64)
    for s in seg_local_real:
        for t in range(TPC):
            a0, a1 = t * P, min((t + 1) * P, SH)
            if a0 >= SH:
                break
            tl = s[a0:a1]
            b0[t] = min(b0[t], int(tl.min()) // B)
            b1[t] = max(b1[t], int(tl.max()) // B)
    assert int((b1 - b0).max()) < 2, "tile spans >2 seg blocks"

    first_b = {}
    last_b = {}
    for t in range(TPC):
        for b in range(int(b0[t]), int(b1[t]) + 1):
            if b not in first_b:
                first_b[b] = t
            last_b[b] = t

    win_first = {}
    win_last = {}
    for b in first_b:
        w = b // 2
        win_first[w] = min(win_first.get(w, 1 << 30), first_b[b])
        win_last[w] = max(win_last.get(w, -1), last_b[b])
    # untouched blocks inside touched windows -> zeroed at flush
    zero_blocks = {
        w: [b for b in range(2 * w, 2 * w + 2) if b not in first_b]
        for w in win_first
    }
    alive = max(
        sum(1 for w in win_first if win_first[w] <= t <= win_last[w])
        for t in range(TPC)
    )
    win_bufs = min(max(2, alive + 1), 4)
    return {
        "NW": NW,
        "b0": [int(v) for v in b0],
        "b1": [int(v) for v in b1],
        "first_b": first_b,
        "last_b": last_b,
        "win_first": win_first,
        "win_last": win_last,
        "zero_blocks": zero_blocks,
        "win_bufs": win_bufs,
    }


def _build(sched, reps=1):
    import contextlib

    import concourse.bacc as bacc
    import concourse.mybir as mybir
    import concourse.tile as tile

    f32 = mybir.dt.float32
    bf16 = mybir.dt.bfloat16
    mdt = mybir.dt.float8e4 if MASK_FP8 else bf16
    NW = sched["NW"]
    starts, sizes = _chunk_plan()
    nchunks = len(starts)

    nc = bacc.Bacc(None, target_bir_lowering=False)
    xp_d = nc.declare_dram_parameter("xp", [nchunks * P, CH * DF], bf16, isOutput=False)
    segs_d = nc.declare_dram_parameter("segs", [P, P + TPC], f32, isOutput=False)
    wk_d = nc.declare_dram_parameter("wk", [P, 8 * D_OUT], bf16, isOutput=False)
    id_d = nc.declare_dram_parameter("ident", [P, P], bf16, isOutput=False)
    out_d = nc.declare_dram_parameter("out", [NW * 32, D_OUT], f32, isOutput=True)

    with tile.TileContext(nc) as tc:
        with (
            tc.tile_pool(name="const", bufs=1) as constp,
            tc.tile_pool(name="chunk", bufs=CHUNK_BUFS) as chunkp,
            tc.tile_pool(name="onehot", bufs=M_BUFS) as mp,
            tc.tile_pool(name="atmp", bufs=AT_BUFS) as atp,
            tc.tile_pool(name="tks", bufs=1) as tkp,
            tc.tile_pool(name="win", bufs=sched["win_bufs"], space="PSUM") as winp,
            tc.tile_pool(name="tp", bufs=TP_BUFS, space="PSUM") as tpp,
            tc.tile_pool(name="po", bufs=PO_BUFS, space="PSUM") as pop,
        ):
            # f32 iota columns 0:P, segs columns P:P+TPC; split into a small
            # fast-completing first DMA (iota + first 32 tile columns, Sync)
            # and the rest on Scalar, so the mask pipeline starts ~1us sooner
            segio_sb = constp.tile([P, P + TPC], f32)
            nc.sync.dma_start(segio_sb[:, : P + 32], segs_d[:, : P + 32])
            nc.scalar.dma_start(segio_sb[:, P + 32 :], segs_d[:, P + 32 :])
            segs_sb = segio_sb
            iota_bf = constp.tile([P, P], bf16)
            nc.vector.tensor_copy(iota_bf[:], segio_sb[:, :P])
            iota_sb = iota_bf[:]
            ident_sb = constp.tile([P, P], bf16)
            wk_sb = constp.tile([P, 8 * D_OUT], bf16)
            zmask_sb = constp.tile([P, B], mdt)
            scratch_sb = constp.tile([1, 1], f32)

            tk0 = tkp.tile([P, NW * P], bf16, tag="tk0")
            tk1 = tkp.tile([P, NW * P], bf16, tag="tk1")

            loop_cm = (
                tc.For_i(
                    0,
                    reps,
                    1,
                    hint_engines=(
                        mybir.EngineType.PE,
                        mybir.EngineType.DVE,
                        mybir.EngineType.Activation,
                        mybir.EngineType.SP,
                    ),
                )
                if reps > 1
                else contextlib.nullcontext()
            )
            first_body = [True]
            with loop_cm:
                _emit_body(
                    nc, tc, mybir, f32, bf16, mdt, sched, starts, sizes,
                    chunkp, mp, atp, winp, tpp, pop,
                    segs_sb, iota_sb, ident_sb, wk_sb, zmask_sb,
                    scratch_sb, tk0, tk1, xp_d, out_d, id_d, wk_d,
                    first_body,
                )

    nc.compile()
    return nc


def _emit_body(
    nc, tc, mybir, f32, bf16, mdt, sched, starts, sizes,
    chunkp, mp, atp, winp, tpp, pop,
    segs_sb, iota_sb, ident_sb, wk_sb, zmask_sb,
    scratch_sb, tk0, tk1, xp_d, out_d, id_d, wk_d, first_body,
):
    NW = sched["NW"]
    NWG = NW // 4
    b0 = sched["b0"]
    b1 = sched["b1"]
    first_b = sched["first_b"]
    last_b = sched["last_b"]
    win_last = sched["win_last"]
    zero_blocks = sched["zero_blocks"]

    po_tiles = {}
    po_done = {g: 0 for g in range(NWG)}
    pairs_done = set()

    def emit_stage2_pair(w_lo):
        # windows (w_lo, w_lo+1) fill output partitions [64r, 64r+64) of
        # group g's psum, r = (w_lo//2) % 2 (PE out base must be 0/32/64)
        g, r = w_lo // 4, (w_lo // 2) % 2
        pairs_done.add(w_lo)
        if g not in po_tiles:
            po_tiles[g] = pop.tile([P, D_OUT], f32, tag="po", name=f"po{g}")
        po = po_tiles[g]
        blk = po[64 * r : 64 * r + 64, :]
        for kc, tkbuf in ((0, tk0), (1, tk1)):
            for s in range(N_SPECIES):
                nc.tensor.matmul(
                    blk,
                    lhsT=tkbuf[:, w_lo * P + s : (w_lo + 2) * P : 4],
                    rhs=wk_sb[:, (s * 2 + kc) * D_OUT : (s * 2 + kc + 1) * D_OUT],
                    start=(kc == 0 and s == 0),
                    stop=(kc == 1 and s == N_SPECIES - 1),
                )
        po_done[g] += 1
        if po_done[g] == 2:
            ob = atp.tile([P, D_OUT], f32, tag="ob", name=f"ob{g}")
            nc.scalar.copy(ob[:], po[:])
            nc.sync.dma_start(out_d[g * P : (g + 1) * P, :], ob[:])
            del po_tiles[g]

    psw = {}
    chunk = None
    coff = 0
    ci = -1
    for t in range(TPC):
        if t in sizes:
            ci += 1
            csz = sizes[t]
            chunk = chunkp.tile([P, CH * DF], bf16, tag="chunk", name=f"ch{t}")
            eng = nc.sync if ci % 2 == 0 else nc.scalar
            eng.dma_start(
                chunk[:, : csz * DF], xp_d[ci * P : (ci + 1) * P, : csz * DF]
            )
            coff = t
            if first_body[0]:
                if ci == 1:
                    # gpsimd is otherwise idle: zero-fill tk + zmask there
                    nc.gpsimd.memset(tk0[:], 0.0)
                    nc.gpsimd.memset(tk1[:], 0.0)
                    nc.gpsimd.memset(zmask_sb[:], 0.0)
                elif ci == 2:
                    nc.sync.dma_start(ident_sb[:], id_d[:])
                elif ci == 3:
                    # trigger the scalar-engine act table load early (1.3us)
                    # so it doesn't stall the first window flush
                    nc.scalar.copy(scratch_sb[:], segs_sb[:1, :1])
                elif ci == 4:
                    nc.sync.dma_start(wk_sb[:], wk_d[:])
                    first_body[0] = False
        xt = chunk[:, (t - coff) * DF : (t - coff + 1) * DF]
        nblk = b1[t] - b0[t] + 1
        m = mp.tile([P, P], mdt, tag="m")
        # m[a, j] = (iota[j] - seg[a] == -64*b0)  <=>  seg[a] == 64*b0 + j
        nc.vector.tensor_scalar(
            out=m[:, : nblk * B],
            in0=iota_sb[:, : nblk * B],
            scalar1=segs_sb[:, P + t : P + t + 1],
            scalar2=float(-(B * b0[t])),
            op0=mybir.AluOpType.subtract,
            op1=mybir.AluOpType.is_equal,
        )
        for b in range(b0[t], b1[t] + 1):
            w = b // 2
            if w not in psw:
                psw[w] = winp.tile([P, DF], f32, tag="win", name=f"win{w}")
            base = B * (b % 2)
            nc.tensor.matmul(
                psw[w][base : base + B, :],
                lhsT=m[:, (b - b0[t]) * B : (b - b0[t] + 1) * B],
                rhs=xt,
                start=(t == first_b[b]),
                stop=(t == last_b[b]),
            )
        # flush finished windows: transpose into feature-major buffers
        for w in sorted(psw):
            if t != win_last[w]:
                continue
            for b in zero_blocks[w]:
                base = B * (b % 2)
                nc.tensor.matmul(
                    psw[w][base : base + B, :],
                    lhsT=zmask_sb[:],
                    rhs=xt,
                    start=True,
                    stop=True,
                )
            at = atp.tile([P, DF], bf16, tag="at")
            nc.scalar.copy(at[:], psw[w][:])
            for kc, tkbuf in ((0, tk0), (1, tk1)):
                tp = tpp.tile([P, P], bf16, tag="tp")
                nc.tensor.transpose(
                    out=tp[:],
                    in_=at[:, kc * P : (kc + 1) * P],
                    identity=ident_sb[:],
                )
                nc.scalar.copy(tkbuf[:, w * P : (w + 1) * P], tp[:])
            del psw[w]
            # stage 2 for a window pair once its later window is flushed
            if w % 2 == 1:
                emit_stage2_pair(w - 1)

    # remaining pairs (NW padding / odd tail): zeros via memset tk columns
    for w_lo in range(0, NW, 2):
        if w_lo not in pairs_done:
            emit_stage2_pair(w_lo)


def _prep(x, W, b, central_species, structural_indices):
    """Host-side prep: schedule from indices + packed per-core in_maps."""
    import ml_dtypes

    bf16 = ml_dtypes.bfloat16
    x = np.asarray(x, dtype=np.float32)
    Wf = np.asarray(W, dtype=np.float32)
    bf = np.asarray(b, dtype=np.float32)
    cs = np.asarray(central_species).astype(np.int64)
    si = np.asarray(structural_indices).astype(np.int64)

    if not np.all(np.diff(si) >= 0):
        order = np.argsort(si, kind="stable")
        si = si[order]
        cs = cs[order]
        x = x[order]

    seg = 4 * si + cs
    # host-side bias term: sum over atoms of b[species], per structure
    counts = np.bincount(seg, minlength=4 * N_STRUCT).reshape(N_STRUCT, 4)
    bias_full = counts.astype(np.float32) @ bf
    g0 = [int(si[c * SH]) for c in range(N_CORES)]
    seg_local_real = [
        (seg[c * SH : (c + 1) * SH] - 4 * g0[c]).astype(np.int64)
        for c in range(N_CORES)
    ]
    sched = _schedule(seg_local_real)
    starts, sizes = _chunk_plan()

    iota = np.tile(np.arange(P, dtype=np.float32), (P, 1))
    ident = np.eye(P, dtype=bf16)
    wk = np.zeros((P, 8, D_OUT), bf16)
    for s in range(N_SPECIES):
        for kc in range(2):
            wk[:, s * 2 + kc, :] = Wf[s, kc * P : (kc + 1) * P, :].astype(bf16)
    wk = np.ascontiguousarray(wk.reshape(P, 8 * D_OUT))

    in_maps = []
    for c in range(N_CORES):
        xp = np.zeros((SH_PAD, DF), bf16)
        xp[:SH] = x[c * SH : (c + 1) * SH].astype(bf16)
        # partition-major within each chunk; chunks are contiguous DRAM
        # blocks so every DMA src is one linear region
        xp = xp.reshape(TPC, P, DF)
        xpk = np.zeros((len(starts) * P, CH * DF), bf16)
        for ci, t0 in enumerate(starts):
            csz = sizes[t0]
            blk = xp[t0 : t0 + csz].transpose(1, 0, 2).reshape(P, csz * DF)
            xpk[ci * P : (ci + 1) * P, : csz * DF] = blk
        segsT = np.full((TPC, P), SENTINEL, np.float32)
        segsT.reshape(-1)[:SH] = seg_local_real[c].astype(np.float32)
        segsT = np.ascontiguousarray(np.concatenate([iota, segsT.T], axis=1))
        in_maps.append(
            {"xp": xpk, "segs": segsT, "wk": wk, "ident": ident}
        )
    return {
        "build_args": (sched,),
        "in_maps": in_maps,
        "g0": g0,
        "NW": sched["NW"],
        "bias_full": bias_full,
    }


def kernel(x, W, b, central_species, structural_indices):
    from concourse.bass_utils import run_bass_kernel_spmd

    prep = _prep(x, W, b, central_species, structural_indices)
    nc = _build(*prep["build_args"])
    res = run_bass_kernel_spmd(
        nc, prep["in_maps"], core_ids=list(range(N_CORES))
    )

    g0, NW = prep["g0"], prep["NW"]
    full = np.zeros((N_STRUCT + NW * 32, D_OUT), np.float32)
    for c in range(N_CORES):
        full[g0[c] : g0[c] + NW * 32] += res.results[c]["out"]
    out = full[:N_STRUCT] + prep["bias_full"]
    return np.ascontiguousarray(out)
